# revision 14
# baseline (speedup 1.0000x reference)
"""Dissipative Hamiltonian derivation — Trainium2 Bass kernel, 8-core SPMD.

Math (closed-form gradients, identical derivation to the validated baseline):
  vs = sigmoid(v); vq = [vs, q]; R = vq @ W1_w.T; U = R + b
  S[i,j] = ||u_j - r_i||^2 ;  d = softplus(S) = S + ln(1+exp(-S))
  sigmoid(S) = 1 - exp(-d)   (so the whole chain runs off ONE ACT table:
                              natural_log_exp, no table thrash)
  C[i,j] = 2*mask[i,j]*(d-2)*d^-3*sigmoid(S),  mask = (mvw*m).T@(mvw*m)
  B[i] = (C @ [U|1])[i]      (row-local)
  P[j] = (C.T @ [R|1])[j]    (AllToAll + DMA-accumulate gather = cross-core sum)
  A[j] = colsum(C)[j]*u_j - (C.T R)[j]
  dHdq = (A - B) @ W1_w[:, 64:]   (diag of C cancels exactly in A - B)
  dp = -(dHdq + dissipated)  with the dissipated/kinetic branches computed on
  the host (they depend only on v,p,m — not on the N^2 pairwise part), as is
  dq = dHdp entirely.

Device timeline per core (192 of 1536 rows):
  S matmuls (k=18, f=512) -> batched ACT chain -> C (bf16) -> P chunk matmuls
  -> DRAM -> AllToAll; B transposes+accum run inside the collective window;
  tail: DMA-accumulate the 8 P slabs, assemble D, one k=16 matmul, dp out.
"""

import os
import numpy as np

N = 1536
NCORES = 8
SH = N // NCORES            # 192 rows per core
H = 16
VD = 64
ITILES = [(0, 128), (128, 64)]   # i-tiles inside a shard (partition dim <= 128)
NJ = N // 128                # 12 j-chunks of 128
NJ3 = N // 512               # 3 j-chunks of 512

_CACHE = {}


def _build_nc():
    from concourse import bacc, mybir
    import concourse.tile as tile

    f32 = mybir.dt.float32
    bf16 = mybir.dt.bfloat16
    AF = mybir.ActivationFunctionType
    OP = mybir.AluOpType

    nc = bacc.Bacc(None, num_devices=NCORES)

    def ein(name, shape, dt=f32):
        return nc.dram_tensor(name, shape, dt, kind="ExternalInput")

    Srhs_d = ein("Srhs", [18, N])        # [U.T; ones; un2] replicated
    Slhs_d = ein("Slhs", [18, SH])       # [-2 R_s.T; rn2_s; ones]
    mvwm_d = ein("mvwm", [48, N], bf16)  # mvw * m, replicated
    mvwms2_d = ein("mvwms2", [48, SH], bf16)   # -2 * (mvw*m) shard cols
    uro_d = ein("uro", [128, 17 * NJ], bf16)   # [U|1] rows per 128-chunk
    rro0_d = ein("rro0", [128, 17], bf16)      # [R_s|1] rows
    rro1_d = ein("rro1", [64, 17], bf16)
    urs0_d = ein("urs0", [128, H])       # U_s rows fp32 (assembly)
    urs1_d = ein("urs1", [64, H])
    rrs0_d = ein("rrs0", [128, H])       # R_s rows fp32 (assembly)
    rrs1_d = ein("rrs1", [64, H])
    dds0_d = ein("dds0", [128, 32])      # host-computed dissipated term
    dds1_d = ein("dds1", [64, 32])
    W1q_d = ein("W1q", [H, 32])
    identb_d = ein("identb", [128, 128], bf16)
    identf_d = ein("identf", [128, 128])

    dp_d = nc.dram_tensor("dp_s", [SH, 32], f32, kind="ExternalOutput")

    with tile.TileContext(nc) as tc:
        with (
            tc.tile_pool(name="const", bufs=1) as cp,
            tc.tile_pool(name="work", bufs=3) as wp,
            tc.tile_pool(name="big", bufs=1) as wp1,
            tc.tile_pool(name="dram", bufs=1, space="DRAM") as drp,
        ):
            def load(eng, d, shape, tag, dt=f32):
                t = cp.tile(shape, dt, tag=tag)
                eng.dma_start(t[:], d[:])
                return t

            # inputs spread across engine DMA queues, first-use first
            Slhs = load(nc.scalar, Slhs_d, [18, SH], "Slhs")
            Srhs = cp.tile([18, N], f32, tag="Srhs")
            for k in range(NJ3):
                nc.sync.dma_start(Srhs[:, k * 512:(k + 1) * 512],
                                  Srhs_d[:, k * 512:(k + 1) * 512])
            mvwms2 = load(nc.gpsimd, mvwms2_d, [48, SH], "mvwms2", bf16)
            mvwm = cp.tile([48, N], bf16, tag="mvwm")
            for k in range(NJ3):
                nc.scalar.dma_start(mvwm[:, k * 512:(k + 1) * 512],
                                    mvwm_d[:, k * 512:(k + 1) * 512])
            rro0 = load(nc.gpsimd, rro0_d, [128, 17], "rro0", bf16)
            rro1 = load(nc.gpsimd, rro1_d, [64, 17], "rro1", bf16)
            identb = load(nc.gpsimd, identb_d, [128, 128], "identb", bf16)
            uro = load(nc.gpsimd, uro_d, [128, 17 * NJ], "uro", bf16)
            urs0 = load(nc.gpsimd, urs0_d, [128, H], "urs0")
            urs1 = load(nc.gpsimd, urs1_d, [64, H], "urs1")
            rrs0 = load(nc.gpsimd, rrs0_d, [128, H], "rrs0")
            rrs1 = load(nc.gpsimd, rrs1_d, [64, H], "rrs1")
            dds0 = load(nc.gpsimd, dds0_d, [128, 32], "dds0")
            dds1 = load(nc.gpsimd, dds1_d, [64, 32], "dds1")
            W1q = load(nc.gpsimd, W1q_d, [H, 32], "W1q")
            identf = load(nc.gpsimd, identf_d, [128, 128], "identf")

            # C tiles, bf16, [i, j] layout
            ct0 = cp.tile([128, N], bf16, tag="ct0")
            ct1 = cp.tile([64, N], bf16, tag="ct1")

            P_dram = drp.tile([N, 17], f32)
            P_out = drp.tile([N, 17], f32)

            TILES = [(it, off, w, k) for k in range(NJ3)
                     for it, (off, w) in enumerate(ITILES)]

            # PSUM pools hand out full 2KB banks; sub-tiles are sliced out so
            # no extra banks are burned per distinct shape. 2+1+1+2+2 = 8.
            with (
                tc.tile_pool(name="psS", bufs=2, space="PSUM") as psS,
                tc.tile_pool(name="psM", bufs=1, space="PSUM") as psM,
                tc.tile_pool(name="psP", bufs=1, space="PSUM") as psP,
                tc.tile_pool(name="psT", bufs=2, space="PSUM") as psT,
                tc.tile_pool(name="psB", bufs=2, space="PSUM") as psB,
            ):
                # ---- PE: S matmuls; copy S to SBUF so the 2 PSUM banks rotate
                S_sb = []
                for it, off, w, k in TILES:
                    sb_ = psS.tile([128, 512], f32, tag="s")
                    nc.tensor.matmul(sb_[0:w, 0:512], Slhs[:, off:off + w],
                                     Srhs[:, k * 512:(k + 1) * 512],
                                     start=True, stop=True)
                    ss = wp1.tile([w, 512], f32, tag=f"ss{it}{k}")
                    nc.vector.tensor_copy(ss[:], sb_[0:w, 0:512])
                    S_sb.append(ss)

                # ---- PE: mask matmuls (bf16; PSUM-resident until sm) ----
                mk_ps = []
                for it, off, w, k in TILES:
                    mb_ = psM.tile([128, 512], f32, tag="m")
                    nc.tensor.matmul(mb_[0:w, 0:512], mvwms2[:, off:off + w],
                                     mvwm[:, k * 512:(k + 1) * 512],
                                     start=True, stop=True)
                    mk_ps.append(mb_[0:w, 0:512])

                # ---- ACT chain, all on the natural_log_exp table ----
                e_sb = []
                for n_, (it, off, w, k) in enumerate(TILES):
                    ee = wp1.tile([w, 512], f32, tag=f"ee{it}{k}")
                    nc.scalar.activation(ee[:], S_sb[n_][:], AF.Exp, scale=-1.0)
                    e_sb.append(ee)
                l_sb = []
                for n_, (it, off, w, k) in enumerate(TILES):
                    ll = wp1.tile([w, 512], f32, tag=f"ll{it}{k}")
                    nc.scalar.activation(ll[:], e_sb[n_][:], AF.Ln, bias=1.0)
                    l_sb.append(ll)
                # DVE: d = S + ln(1+exp(-S))  (exact softplus; S >= 0)
                d_sb = []
                for n_, (it, off, w, k) in enumerate(TILES):
                    dd = wp1.tile([w, 512], f32, tag=f"dd{it}{k}")
                    nc.vector.tensor_add(dd[:], S_sb[n_][:], l_sb[n_][:])
                    d_sb.append(dd)
                # em = exp(-d) = 1 - sigmoid(S)
                em_sb = []
                for n_, (it, off, w, k) in enumerate(TILES):
                    em = wp1.tile([w, 512], f32, tag=f"em{it}{k}")
                    nc.scalar.activation(em[:], d_sb[n_][:], AF.Exp, scale=-1.0)
                    em_sb.append(em)
                # sm = (em - 1) * mask = sigmoid(S) * |mask|  (mask shipped
                # negated, so the sign comes out right)
                sm_sb = []
                for n_, (it, off, w, k) in enumerate(TILES):
                    sm = wp1.tile([w, 512], f32, tag=f"sm{it}{k}")
                    nc.vector.scalar_tensor_tensor(
                        sm[:], em_sb[n_][:], -1.0, mk_ps[n_],
                        op0=OP.add, op1=OP.mult)
                    sm_sb.append(sm)
                ln_sb = []
                for n_, (it, off, w, k) in enumerate(TILES):
                    ld = wp1.tile([w, 512], f32, tag=f"ld{it}{k}")
                    nc.scalar.activation(ld[:], d_sb[n_][:], AF.Ln)
                    ln_sb.append(ld)
                p3_sb = []
                for n_, (it, off, w, k) in enumerate(TILES):
                    p3 = wp1.tile([w, 512], f32, tag=f"p3{it}{k}")
                    nc.scalar.activation(p3[:], ln_sb[n_][:], AF.Exp, scale=-3.0)
                    p3_sb.append(p3)

                # ---- DVE: C = ((d-2)*d^-3) * sm -> bf16; P matmuls per
                # 128-chunk -> DRAM. B transposes/accum deferred into the
                # collective window. ----
                for k in range(NJ3):
                    for it, (off, w) in enumerate(ITILES):
                        n_ = 2 * k + it
                        ct = (ct0, ct1)[it]
                        t_ = wp.tile([w, 512], f32, tag=f"t{it}")
                        nc.vector.scalar_tensor_tensor(
                            t_[:], d_sb[n_][:], -2.0, p3_sb[n_][:],
                            op0=OP.add, op1=OP.mult)
                        nc.vector.tensor_mul(ct[:, k * 512:(k + 1) * 512],
                                             t_[:], sm_sb[n_][:])
                    for sub in range(4):
                        jc = 4 * k + sub
                        pb_ = psP.tile([128, 512], f32, tag="p")
                        pp = pb_[0:128, 0:17]
                        nc.tensor.matmul(pp, ct0[:, jc * 128:(jc + 1) * 128],
                                         rro0[:], start=True, stop=False)
                        nc.tensor.matmul(pp, ct1[:, jc * 128:(jc + 1) * 128],
                                         rro1[:], start=False, stop=True)
                        psb_ = wp.tile([128, 17], f32, tag="psb")
                        nc.vector.tensor_copy(psb_[:], pp)
                        eng = (nc.sync, nc.gpsimd, nc.sync, nc.gpsimd)[sub]
                        eng.dma_start(P_dram[jc * 128:(jc + 1) * 128, :],
                                      psb_[:])

                nc.gpsimd.collective_compute(
                    "AllToAll",
                    mybir.AluOpType.bypass,
                    replica_groups=[list(range(NCORES))],
                    ins=[P_dram.opt()],
                    outs=[P_out.opt()],
                )

                # ---- B = C @ [U|1] via bf16 PE transposes (inside A2A) ----
                bb0 = psB.tile([128, 512], f32, tag="b")
                bb1 = psB.tile([128, 512], f32, tag="b")
                bp0 = bb0[0:128, 0:17]
                bp1 = bb1[0:64, 0:17]
                for jc in range(NJ):
                    for it, (off, w) in enumerate(ITILES):
                        ct = (ct0, ct1)[it]
                        bp = (bp0, bp1)[it]
                        tb_ = psT.tile([128, 1024], bf16, tag="ct")
                        tp = tb_[0:128, 0:w]
                        nc.tensor.transpose(tp, ct[:, jc * 128:(jc + 1) * 128],
                                            identb[0:w, 0:w])
                        tsbb = wp.tile([128, w], bf16, tag=f"tsbb{it}")
                        nc.vector.tensor_copy(tsbb[:], tp)
                        nc.tensor.matmul(bp, tsbb[:],
                                         uro[:, jc * 17:(jc + 1) * 17],
                                         start=(jc == 0), stop=(jc == NJ - 1))
                bsb = []
                for it, (off, w) in enumerate(ITILES):
                    bs = wp.tile([w, 17], f32, tag=f"bsb{it}")
                    nc.vector.tensor_copy(bs[:], (bp0, bp1)[it])
                    bsb.append(bs)

                # ---- tail: gather + sum the 8 P_out slabs, assemble, dp ----
                # it0: gpsimd software-DGE accumulate; it1: plain loads + DVE
                acc0 = wp.tile([128, 17], f32, tag="acc0")
                for c in range(NCORES):
                    nc.gpsimd.dma_start(
                        acc0[:], P_out[c * SH:c * SH + 128, :],
                        accum_op=(OP.bypass if c == 0 else OP.add))
                acc1w = wp.tile([64, 8 * 17], f32, tag="acc1w")
                for c in range(NCORES):
                    nc.sync.dma_start(
                        acc1w[:, c * 17:(c + 1) * 17],
                        P_out[c * SH + 128:(c + 1) * SH, :])
                for c in range(1, NCORES):
                    nc.vector.tensor_add(acc1w[:, 0:17], acc1w[:, 0:17],
                                         acc1w[:, c * 17:c * 17 + 17])
                for it, (off, w) in enumerate(ITILES):
                    acc = (acc0, acc1w)[it]
                    urs = (urs0, urs1)[it]
                    rrs = (rrs0, rrs1)[it]
                    bs = bsb[it]
                    # A - B = (urs*cc - P16) + (rrs*bc - bs16)
                    a_t = wp.tile([w, H], f32, tag="a_t")
                    nc.vector.scalar_tensor_tensor(
                        a_t[:], urs[:], acc[:, H:H + 1], acc[:, 0:H],
                        op0=OP.mult, op1=OP.subtract)
                    b_t = wp.tile([w, H], f32, tag="b_t")
                    nc.vector.scalar_tensor_tensor(
                        b_t[:], rrs[:], bs[:, H:17], bs[:, 0:H],
                        op0=OP.mult, op1=OP.subtract)
                    d_t = wp.tile([w, H], f32, tag="d_t")
                    nc.vector.tensor_add(d_t[:], a_t[:], b_t[:])
                    mb_ = psM.tile([128, 512], f32, tag="m")
                    dtp = mb_[0:H, 0:w]
                    nc.tensor.transpose(dtp, d_t[:], identf[0:w, 0:w])
                    dts = wp.tile([H, w], f32, tag="dts")
                    nc.vector.tensor_copy(dts[:], dtp)
                    hb_ = psP.tile([128, 512], f32, tag="p")
                    hq = hb_[0:w, 0:32]
                    nc.tensor.matmul(hq, dts[:], W1q[:], start=True, stop=True)
                    dpo = wp.tile([w, 32], f32, tag="dpo")
                    # dp = -(hq + dds) = (hq * -1) - dds
                    nc.vector.scalar_tensor_tensor(
                        dpo[:], hq, -1.0, (dds0, dds1)[it][:],
                        op0=OP.mult, op1=OP.subtract)
                    eng2 = (nc.sync, nc.scalar)[it]
                    eng2.dma_start(dp_d[off:off + w, :], dpo[:])

    nc.finalize()
    return nc


def _prepare_in_maps(v, e, m, p, q, mvw, W_T, W1_w, W1_b, W_F):
    f32 = np.float32
    v, m, p, q, mvw = (np.asarray(x, f32) for x in (v, m, p, q, mvw))
    W_T, W1_w, W1_b, W_F = (np.asarray(x, f32) for x in (W_T, W1_w, W1_b, W_F))
    import ml_dtypes
    bf16 = ml_dtypes.bfloat16

    vs = (1.0 / (1.0 + np.exp(-v))).astype(f32)
    vq = np.concatenate([vs, q], axis=1)                    # [N,96]
    R = (vq @ W1_w.T).astype(f32)                           # [N,16]
    U = (R + W1_b[None, :]).astype(f32)
    rn2 = (R * R).sum(axis=1).astype(f32)
    un2 = (U * U).sum(axis=1).astype(f32)
    ones = np.ones((N,), f32)

    Srhs = np.ascontiguousarray(np.vstack([U.T, ones[None, :], un2[None, :]]))
    Slhs_full = np.vstack([-2.0 * R.T, rn2[None, :], ones[None, :]])
    mvwm = np.ascontiguousarray(mvw * m[:, 0][None, :])              # [48,N]

    # kinetic (dq) and dissipated branches: pure input functions, done here.
    def sp_sig(z):
        pw = np.logaddexp(0.0, z)
        sg = 1.0 / (1.0 + np.exp(-z))
        return pw * sg

    mi2 = 2.0 / m                                           # [N,1]
    zT = np.concatenate([vs, p], axis=1) @ W_T.T            # [N,16]
    dq_full = (mi2 * sp_sig(zT)) @ W_T[:, VD:]              # [N,32]
    zF = p @ W_F.T
    dds_full = (mi2 * sp_sig(zF)) @ W_F                     # [N,32]

    uro = np.empty((128, 17 * NJ), f32)
    for jc in range(NJ):
        uro[:, jc * 17:jc * 17 + H] = U[jc * 128:(jc + 1) * 128, :]
        uro[:, jc * 17 + H] = 1.0

    shared = {
        "Srhs": Srhs,
        "mvwm": mvwm.astype(bf16),
        "uro": uro.astype(bf16),
        "W1q": np.ascontiguousarray(W1_w[:, VD:]),
        "identb": np.eye(128, dtype=f32).astype(bf16),
        "identf": np.eye(128, dtype=f32),
    }
    in_maps = []
    for c in range(NCORES):
        sl = slice(c * SH, (c + 1) * SH)
        Rs, Us = R[sl], U[sl]
        rro = np.empty((SH, 17), f32)
        rro[:, 0:H] = Rs
        rro[:, H] = 1.0
        dds_s = dds_full[sl].astype(f32)
        in_maps.append({
            **shared,
            "Slhs": np.ascontiguousarray(Slhs_full[:, sl]),
            "mvwms2": np.ascontiguousarray(-2.0 * mvwm[:, sl]).astype(bf16),
            "rro0": np.ascontiguousarray(rro[0:128]).astype(bf16),
            "rro1": np.ascontiguousarray(rro[128:]).astype(bf16),
            "urs0": np.ascontiguousarray(Us[0:128]),
            "urs1": np.ascontiguousarray(Us[128:]),
            "rrs0": np.ascontiguousarray(Rs[0:128]),
            "rrs1": np.ascontiguousarray(Rs[128:]),
            "dds0": np.ascontiguousarray(dds_s[0:128]),
            "dds1": np.ascontiguousarray(dds_s[128:]),
        })
    return in_maps, dq_full.astype(f32)


def kernel(v, e, m, p, q, mvw, W_T, W1_w, W1_b, W_F):
    from concourse.bass_utils import run_bass_kernel_spmd

    in_maps, dq_full = _prepare_in_maps(v, e, m, p, q, mvw, W_T, W1_w, W1_b, W_F)

    if "nc" not in _CACHE:
        _CACHE["nc"] = _build_nc()
    nc = _CACHE["nc"]

    trace = bool(os.environ.get("BASS_KERNEL_TRACE"))
    if trace:
        try:
            from antenv.axon_hooks import get_axon_ntff_profile_hook  # noqa: F401
        except ImportError:
            trace = False
    res = run_bass_kernel_spmd(nc, in_maps, list(range(NCORES)), trace=trace)
    if trace and res.exec_time_ns is not None:
        print(f"HW exec time: {res.exec_time_ns} ns")

    dp = np.concatenate([res.results[c]["dp_s"] for c in range(NCORES)], axis=0)
    return dp, dq_full


# revision 15
# speedup vs baseline: 1.1478x; 1.1478x over previous
"""Dissipative Hamiltonian derivation — Trainium2 Bass kernel, 8-core SPMD.

Math (closed-form gradients, identical derivation to the validated baseline):
  vs = sigmoid(v); vq = [vs, q]; R = vq @ W1_w.T; U = R + b
  S[i,j] = ||u_j - r_i||^2 ;  d = softplus(S) = S + ln(1+exp(-S))
  sigmoid(S) = 1 - exp(-d)   (so the whole chain runs off ONE ACT table:
                              natural_log_exp, no table thrash)
  C[i,j] = 2*mask[i,j]*(d-2)*d^-3*sigmoid(S),  mask = (mvw*m).T@(mvw*m)
  B[i] = (C @ [U|1])[i]      (row-local)
  P[j] = (C.T @ [R|1])[j]    (AllToAll + DMA-accumulate gather = cross-core sum)
  A[j] = colsum(C)[j]*u_j - (C.T R)[j]
  dHdq = (A - B) @ W1_w[:, 64:]   (diag of C cancels exactly in A - B)
  dp = -(dHdq + dissipated)  with the dissipated/kinetic branches computed on
  the host (they depend only on v,p,m — not on the N^2 pairwise part), as is
  dq = dHdp entirely.

Device timeline per core (192 of 1536 rows):
  S matmuls (k=18, f=512) -> batched ACT chain -> C (bf16) -> P chunk matmuls
  -> DRAM -> AllToAll; B transposes+accum run inside the collective window;
  tail: DMA-accumulate the 8 P slabs, assemble D, one k=16 matmul, dp out.
"""

import os
import numpy as np

N = 1536
NCORES = 8
SH = N // NCORES            # 192 rows per core
H = 16
VD = 64
ITILES = [(0, 128), (128, 64)]   # i-tiles inside a shard (partition dim <= 128)
NJ = N // 128                # 12 j-chunks of 128
NJ3 = N // 512               # 3 j-chunks of 512

_CACHE = {}


def _build_nc():
    from concourse import bacc, mybir
    import concourse.tile as tile

    f32 = mybir.dt.float32
    f32r = mybir.dt.float32r
    bf16 = mybir.dt.bfloat16
    AF = mybir.ActivationFunctionType
    OP = mybir.AluOpType

    nc = bacc.Bacc(None, num_devices=NCORES)

    def ein(name, shape, dt=f32):
        return nc.dram_tensor(name, shape, dt, kind="ExternalInput")

    Srhs_d = ein("Srhs", [18, N], f32r)  # [U.T; ones; un2] replicated
    Slhs_d = ein("Slhs", [18, SH], f32r) # [-2 R_s.T; rn2_s; ones]
    mvwm_d = ein("mvwm", [48, N], bf16)  # mvw * m, replicated
    mvwms2_d = ein("mvwms2", [48, SH], bf16)   # -2 * (mvw*m) shard cols
    uro_d = ein("uro", [128, 17 * NJ], bf16)   # [U|1] rows per 128-chunk
    rro0_d = ein("rro0", [128, 17], bf16)      # [R_s|1] rows
    rro1_d = ein("rro1", [64, 17], bf16)
    urs0_d = ein("urs0", [128, H])       # U_s rows fp32 (assembly)
    urs1_d = ein("urs1", [64, H])
    rrs0_d = ein("rrs0", [128, H])       # R_s rows fp32 (assembly)
    rrs1_d = ein("rrs1", [64, H])
    dds0_d = ein("dds0", [128, 32])      # host-computed dissipated term
    dds1_d = ein("dds1", [64, 32])
    W1q_d = ein("W1q", [H, 32])
    identb_d = ein("identb", [128, 128], bf16)
    identf_d = ein("identf", [128, 128])

    dp_d = nc.dram_tensor("dp_s", [SH, 32], f32, kind="ExternalOutput")

    with tile.TileContext(nc) as tc:
        with (
            tc.tile_pool(name="const", bufs=1) as cp,
            tc.tile_pool(name="work", bufs=3) as wp,
            tc.tile_pool(name="big", bufs=1) as wp1,
            tc.tile_pool(name="dram", bufs=1, space="DRAM") as drp,
        ):
            def load(eng, d, shape, tag, dt=f32):
                t = cp.tile(shape, dt, tag=tag)
                eng.dma_start(t[:], d[:])
                return t

            # inputs spread across engine DMA queues, first-use first
            Slhs = load(nc.scalar, Slhs_d, [18, SH], "Slhs", f32r)
            Srhs = cp.tile([18, N], f32r, tag="Srhs")
            for k in range(NJ3):
                nc.sync.dma_start(Srhs[:, k * 512:(k + 1) * 512],
                                  Srhs_d[:, k * 512:(k + 1) * 512])
            mvwms2 = load(nc.gpsimd, mvwms2_d, [48, SH], "mvwms2", bf16)
            mvwm = cp.tile([48, N], bf16, tag="mvwm")
            for k in range(NJ3):
                nc.scalar.dma_start(mvwm[:, k * 512:(k + 1) * 512],
                                    mvwm_d[:, k * 512:(k + 1) * 512])
            rro0 = load(nc.gpsimd, rro0_d, [128, 17], "rro0", bf16)
            rro1 = load(nc.gpsimd, rro1_d, [64, 17], "rro1", bf16)
            identb = load(nc.gpsimd, identb_d, [128, 128], "identb", bf16)
            uro = load(nc.gpsimd, uro_d, [128, 17 * NJ], "uro", bf16)
            urs0 = load(nc.gpsimd, urs0_d, [128, H], "urs0")
            urs1 = load(nc.gpsimd, urs1_d, [64, H], "urs1")
            rrs0 = load(nc.gpsimd, rrs0_d, [128, H], "rrs0")
            rrs1 = load(nc.gpsimd, rrs1_d, [64, H], "rrs1")
            dds0 = load(nc.gpsimd, dds0_d, [128, 32], "dds0")
            dds1 = load(nc.gpsimd, dds1_d, [64, 32], "dds1")
            W1q = load(nc.gpsimd, W1q_d, [H, 32], "W1q")
            identf = load(nc.gpsimd, identf_d, [128, 128], "identf")

            # C tiles, bf16, [i, j] layout
            ct0 = cp.tile([128, N], bf16, tag="ct0")
            ct1 = cp.tile([64, N], bf16, tag="ct1")

            P_dram = drp.tile([N, 17], f32)
            P_out = drp.tile([N, 17], f32)

            TILES = [(it, off, w, k) for k in range(NJ3)
                     for it, (off, w) in enumerate(ITILES)]

            # PSUM pools hand out full 2KB banks; sub-tiles are sliced out so
            # no extra banks are burned per distinct shape. 2+1+1+2+2 = 8.
            with (
                tc.tile_pool(name="psS", bufs=2, space="PSUM") as psS,
                tc.tile_pool(name="psM", bufs=1, space="PSUM") as psM,
                tc.tile_pool(name="psP", bufs=1, space="PSUM") as psP,
                tc.tile_pool(name="psT", bufs=2, space="PSUM") as psT,
                tc.tile_pool(name="psB", bufs=2, space="PSUM") as psB,
            ):
                # ---- PE: S matmuls; copy S to SBUF so the 2 PSUM banks rotate
                S_sb = []
                for it, off, w, k in TILES:
                    sb_ = psS.tile([128, 512], f32, tag="s")
                    nc.tensor.matmul(sb_[0:w, 0:512], Slhs[:, off:off + w],
                                     Srhs[:, k * 512:(k + 1) * 512],
                                     start=True, stop=True)
                    ss = wp1.tile([w, 512], f32, tag=f"ss{it}{k}")
                    nc.vector.tensor_copy(ss[:], sb_[0:w, 0:512])
                    S_sb.append(ss)

                # ---- PE: mask matmuls (bf16; PSUM-resident until sm) ----
                mk_ps = []
                for it, off, w, k in TILES:
                    mb_ = psM.tile([128, 512], f32, tag="m")
                    nc.tensor.matmul(mb_[0:w, 0:512], mvwms2[:, off:off + w],
                                     mvwm[:, k * 512:(k + 1) * 512],
                                     start=True, stop=True)
                    mk_ps.append(mb_[0:w, 0:512])

                # ---- ACT chain, all on the natural_log_exp table ----
                e_sb = []
                for n_, (it, off, w, k) in enumerate(TILES):
                    ee = wp1.tile([w, 512], f32, tag=f"ee{it}{k}")
                    nc.scalar.activation(ee[:], S_sb[n_][:], AF.Exp, scale=-1.0)
                    e_sb.append(ee)
                l_sb = []
                for n_, (it, off, w, k) in enumerate(TILES):
                    ll = wp1.tile([w, 512], f32, tag=f"ll{it}{k}")
                    nc.scalar.activation(ll[:], e_sb[n_][:], AF.Ln, bias=1.0)
                    l_sb.append(ll)
                # DVE: d = S + ln(1+exp(-S))  (exact softplus; S >= 0)
                d_sb = []
                for n_, (it, off, w, k) in enumerate(TILES):
                    dd = wp1.tile([w, 512], f32, tag=f"dd{it}{k}")
                    nc.vector.tensor_add(dd[:], S_sb[n_][:], l_sb[n_][:])
                    d_sb.append(dd)
                # lnd batch directly follows the l1 Ln batch (same table)
                ln_sb = []
                for n_, (it, off, w, k) in enumerate(TILES):
                    ld = wp1.tile([w, 512], f32, tag=f"ld{it}{k}")
                    nc.scalar.activation(ld[:], d_sb[n_][:], AF.Ln)
                    ln_sb.append(ld)
                # em = exp(-d) = 1 - sigmoid(S); p3 = d^-3 (one Exp batch)
                em_sb = []
                for n_, (it, off, w, k) in enumerate(TILES):
                    em = wp1.tile([w, 512], f32, tag=f"em{it}{k}")
                    nc.scalar.activation(em[:], d_sb[n_][:], AF.Exp, scale=-1.0)
                    em_sb.append(em)
                p3_sb = []
                for n_, (it, off, w, k) in enumerate(TILES):
                    p3 = wp1.tile([w, 512], f32, tag=f"p3{it}{k}")
                    nc.scalar.activation(p3[:], ln_sb[n_][:], AF.Exp, scale=-3.0)
                    p3_sb.append(p3)
                # sm = (em - 1) * mask = sigmoid(S) * |mask|  (mask shipped
                # negated, so the sign comes out right)
                sm_sb = []
                for n_, (it, off, w, k) in enumerate(TILES):
                    sm = wp1.tile([w, 512], f32, tag=f"sm{it}{k}")
                    nc.vector.scalar_tensor_tensor(
                        sm[:], em_sb[n_][:], -1.0, mk_ps[n_],
                        op0=OP.add, op1=OP.mult)
                    sm_sb.append(sm)

                # ---- DVE: C = ((d-2)*d^-3) * sm -> bf16; P matmuls per
                # 128-chunk -> DRAM. B transposes/accum deferred into the
                # collective window. ----
                for k in range(NJ3):
                    for it, (off, w) in enumerate(ITILES):
                        n_ = 2 * k + it
                        ct = (ct0, ct1)[it]
                        t_ = wp.tile([w, 512], f32, tag=f"t{it}")
                        nc.vector.scalar_tensor_tensor(
                            t_[:], d_sb[n_][:], -2.0, p3_sb[n_][:],
                            op0=OP.add, op1=OP.mult)
                        nc.vector.tensor_mul(ct[:, k * 512:(k + 1) * 512],
                                             t_[:], sm_sb[n_][:])
                    for sub in range(4):
                        jc = 4 * k + sub
                        pb_ = psP.tile([128, 512], f32, tag="p")
                        pp = pb_[0:128, 0:17]
                        nc.tensor.matmul(pp, ct0[:, jc * 128:(jc + 1) * 128],
                                         rro0[:], start=True, stop=False)
                        nc.tensor.matmul(pp, ct1[:, jc * 128:(jc + 1) * 128],
                                         rro1[:], start=False, stop=True)
                        psb_ = wp.tile([128, 17], f32, tag="psb")
                        nc.vector.tensor_copy(psb_[:], pp)
                        eng = (nc.sync, nc.gpsimd, nc.sync, nc.gpsimd)[sub]
                        eng.dma_start(P_dram[jc * 128:(jc + 1) * 128, :],
                                      psb_[:])

                nc.gpsimd.collective_compute(
                    "AllToAll",
                    mybir.AluOpType.bypass,
                    replica_groups=[list(range(NCORES))],
                    ins=[P_dram.opt()],
                    outs=[P_out.opt()],
                )

                # ---- B = C @ [U|1] via bf16 PE transposes (inside A2A) ----
                bb0 = psB.tile([128, 512], f32, tag="b")
                bb1 = psB.tile([128, 512], f32, tag="b")
                bp0 = bb0[0:128, 0:17]
                bp1 = bb1[0:64, 0:17]
                for jc in range(NJ):
                    for it, (off, w) in enumerate(ITILES):
                        ct = (ct0, ct1)[it]
                        bp = (bp0, bp1)[it]
                        tb_ = psT.tile([128, 1024], bf16, tag="ct")
                        tp = tb_[0:128, 0:w]
                        nc.tensor.transpose(tp, ct[:, jc * 128:(jc + 1) * 128],
                                            identb[0:w, 0:w])
                        tsbb = wp.tile([128, w], bf16, tag=f"tsbb{it}")
                        nc.vector.tensor_copy(tsbb[:], tp)
                        nc.tensor.matmul(bp, tsbb[:],
                                         uro[:, jc * 17:(jc + 1) * 17],
                                         start=(jc == 0), stop=(jc == NJ - 1))
                bsb = []
                for it, (off, w) in enumerate(ITILES):
                    bs = wp.tile([w, 17], f32, tag=f"bsb{it}")
                    nc.vector.tensor_copy(bs[:], (bp0, bp1)[it])
                    bsb.append(bs)

                # ---- tail: gather + tree-sum the 8 P_out slabs, assemble ----
                acc0 = wp.tile([128, 8 * 17], f32, tag="acc0")
                acc1w = wp.tile([64, 8 * 17], f32, tag="acc1w")
                for c in range(NCORES):
                    nc.sync.dma_start(
                        acc0[:, c * 17:(c + 1) * 17],
                        P_out[c * SH:c * SH + 128, :])
                    nc.scalar.dma_start(
                        acc1w[:, c * 17:(c + 1) * 17],
                        P_out[c * SH + 128:(c + 1) * SH, :])
                for c in range(1, NCORES):
                    nc.vector.tensor_add(acc0[:, 0:17], acc0[:, 0:17],
                                         acc0[:, c * 17:c * 17 + 17])
                    nc.vector.tensor_add(acc1w[:, 0:17], acc1w[:, 0:17],
                                         acc1w[:, c * 17:c * 17 + 17])
                for it, (off, w) in enumerate(ITILES):
                    acc = (acc0, acc1w)[it]
                    urs = (urs0, urs1)[it]
                    rrs = (rrs0, rrs1)[it]
                    bs = bsb[it]
                    # A - B = (urs*cc - P16) + (rrs*bc - bs16)
                    a_t = wp.tile([w, H], f32, tag="a_t")
                    nc.vector.scalar_tensor_tensor(
                        a_t[:], urs[:], acc[:, H:H + 1], acc[:, 0:H],
                        op0=OP.mult, op1=OP.subtract)
                    b_t = wp.tile([w, H], f32, tag="b_t")
                    nc.vector.scalar_tensor_tensor(
                        b_t[:], rrs[:], bs[:, H:17], bs[:, 0:H],
                        op0=OP.mult, op1=OP.subtract)
                    d_t = wp.tile([w, H], f32, tag="d_t")
                    nc.vector.tensor_add(d_t[:], a_t[:], b_t[:])
                    mb_ = psM.tile([128, 512], f32, tag="m")
                    dtp = mb_[0:H, 0:w]
                    nc.tensor.transpose(dtp, d_t[:], identf[0:w, 0:w])
                    dts = wp.tile([H, w], f32, tag="dts")
                    nc.vector.tensor_copy(dts[:], dtp)
                    hb_ = psP.tile([128, 512], f32, tag="p")
                    hq = hb_[0:w, 0:32]
                    nc.tensor.matmul(hq, dts[:], W1q[:], start=True, stop=True)
                    dpo = wp.tile([w, 32], f32, tag="dpo")
                    # dp = -(hq + dds) = (hq * -1) - dds
                    nc.vector.scalar_tensor_tensor(
                        dpo[:], hq, -1.0, (dds0, dds1)[it][:],
                        op0=OP.mult, op1=OP.subtract)
                    eng2 = (nc.sync, nc.scalar)[it]
                    eng2.dma_start(dp_d[off:off + w, :], dpo[:])

    nc.finalize()
    return nc


def _prepare_in_maps(v, e, m, p, q, mvw, W_T, W1_w, W1_b, W_F):
    f32 = np.float32
    v, m, p, q, mvw = (np.asarray(x, f32) for x in (v, m, p, q, mvw))
    W_T, W1_w, W1_b, W_F = (np.asarray(x, f32) for x in (W_T, W1_w, W1_b, W_F))
    import ml_dtypes
    bf16 = ml_dtypes.bfloat16

    vs = (1.0 / (1.0 + np.exp(-v))).astype(f32)
    vq = np.concatenate([vs, q], axis=1)                    # [N,96]
    R = (vq @ W1_w.T).astype(f32)                           # [N,16]
    U = (R + W1_b[None, :]).astype(f32)
    rn2 = (R * R).sum(axis=1).astype(f32)
    un2 = (U * U).sum(axis=1).astype(f32)
    ones = np.ones((N,), f32)

    Srhs = np.ascontiguousarray(np.vstack([U.T, ones[None, :], un2[None, :]]))
    Slhs_full = np.vstack([-2.0 * R.T, rn2[None, :], ones[None, :]])
    mvwm = np.ascontiguousarray(mvw * m[:, 0][None, :])              # [48,N]

    # kinetic (dq) and dissipated branches: pure input functions, done here.
    def sp_sig(z):
        pw = np.logaddexp(0.0, z)
        sg = 1.0 / (1.0 + np.exp(-z))
        return pw * sg

    mi2 = 2.0 / m                                           # [N,1]
    zT = np.concatenate([vs, p], axis=1) @ W_T.T            # [N,16]
    dq_full = (mi2 * sp_sig(zT)) @ W_T[:, VD:]              # [N,32]
    zF = p @ W_F.T
    dds_full = (mi2 * sp_sig(zF)) @ W_F                     # [N,32]

    uro = np.empty((128, 17 * NJ), f32)
    for jc in range(NJ):
        uro[:, jc * 17:jc * 17 + H] = U[jc * 128:(jc + 1) * 128, :]
        uro[:, jc * 17 + H] = 1.0

    shared = {
        "Srhs": Srhs,
        "mvwm": mvwm.astype(bf16),
        "uro": uro.astype(bf16),
        "W1q": np.ascontiguousarray(W1_w[:, VD:]),
        "identb": np.eye(128, dtype=f32).astype(bf16),
        "identf": np.eye(128, dtype=f32),
    }
    in_maps = []
    for c in range(NCORES):
        sl = slice(c * SH, (c + 1) * SH)
        Rs, Us = R[sl], U[sl]
        rro = np.empty((SH, 17), f32)
        rro[:, 0:H] = Rs
        rro[:, H] = 1.0
        dds_s = dds_full[sl].astype(f32)
        in_maps.append({
            **shared,
            "Slhs": np.ascontiguousarray(Slhs_full[:, sl]),
            "mvwms2": np.ascontiguousarray(-2.0 * mvwm[:, sl]).astype(bf16),
            "rro0": np.ascontiguousarray(rro[0:128]).astype(bf16),
            "rro1": np.ascontiguousarray(rro[128:]).astype(bf16),
            "urs0": np.ascontiguousarray(Us[0:128]),
            "urs1": np.ascontiguousarray(Us[128:]),
            "rrs0": np.ascontiguousarray(Rs[0:128]),
            "rrs1": np.ascontiguousarray(Rs[128:]),
            "dds0": np.ascontiguousarray(dds_s[0:128]),
            "dds1": np.ascontiguousarray(dds_s[128:]),
        })
    return in_maps, dq_full.astype(f32)


def kernel(v, e, m, p, q, mvw, W_T, W1_w, W1_b, W_F):
    from concourse.bass_utils import run_bass_kernel_spmd

    in_maps, dq_full = _prepare_in_maps(v, e, m, p, q, mvw, W_T, W1_w, W1_b, W_F)

    if "nc" not in _CACHE:
        _CACHE["nc"] = _build_nc()
    nc = _CACHE["nc"]

    trace = bool(os.environ.get("BASS_KERNEL_TRACE"))
    if trace:
        try:
            from antenv.axon_hooks import get_axon_ntff_profile_hook  # noqa: F401
        except ImportError:
            trace = False
    res = run_bass_kernel_spmd(nc, in_maps, list(range(NCORES)), trace=trace)
    if trace and res.exec_time_ns is not None:
        print(f"HW exec time: {res.exec_time_ns} ns")

    dp = np.concatenate([res.results[c]["dp_s"] for c in range(NCORES)], axis=0)
    return dp, dq_full


# revision 16
# speedup vs baseline: 1.2607x; 1.0983x over previous
"""Dissipative Hamiltonian derivation — Trainium2 Bass kernel, 8-core SPMD.

Math (closed-form gradients, identical derivation to the validated baseline):
  vs = sigmoid(v); vq = [vs, q]; R = vq @ W1_w.T; U = R + b
  S[i,j] = ||u_j - r_i||^2 ;  d = softplus(S) = S + ln(1+exp(-S))
  sigmoid(S) = 1 - exp(-d)   (so the whole chain runs off ONE ACT table:
                              natural_log_exp, no table thrash)
  C[i,j] = 2*mask[i,j]*(d-2)*d^-3*sigmoid(S),  mask = (mvw*m).T@(mvw*m)
  B[i] = (C @ [U|1])[i]      (row-local)
  P[j] = (C.T @ [R|1])[j]    (AllToAll + DMA-accumulate gather = cross-core sum)
  A[j] = colsum(C)[j]*u_j - (C.T R)[j]
  dHdq = (A - B) @ W1_w[:, 64:]   (diag of C cancels exactly in A - B)
  dp = -(dHdq + dissipated)  with the dissipated/kinetic branches computed on
  the host (they depend only on v,p,m — not on the N^2 pairwise part), as is
  dq = dHdp entirely.

Device timeline per core (192 of 1536 rows):
  S matmuls (k=18, f=512) -> batched ACT chain -> C (bf16) -> P chunk matmuls
  -> DRAM -> AllToAll; B transposes+accum run inside the collective window;
  tail: DMA-accumulate the 8 P slabs, assemble D, one k=16 matmul, dp out.
"""

import os
import numpy as np

N = 1536
NCORES = 8
SH = N // NCORES            # 192 rows per core
H = 16
VD = 64
ITILES = [(0, 128), (128, 64)]   # i-tiles inside a shard (partition dim <= 128)
NJ = N // 128                # 12 j-chunks of 128
NJ3 = N // 512               # 3 j-chunks of 512

_CACHE = {}


def _build_nc():
    from concourse import bacc, mybir
    import concourse.tile as tile

    f32 = mybir.dt.float32
    f32r = mybir.dt.float32r
    bf16 = mybir.dt.bfloat16
    AF = mybir.ActivationFunctionType
    OP = mybir.AluOpType

    nc = bacc.Bacc(None, num_devices=NCORES)

    def ein(name, shape, dt=f32):
        return nc.dram_tensor(name, shape, dt, kind="ExternalInput")

    Srhs_d = ein("Srhs", [18, N], f32r)  # [U.T; ones; un2] replicated
    Slhs_d = ein("Slhs", [18, SH], f32r) # [-2 R_s.T; rn2_s; ones]
    mvwm_d = ein("mvwm", [48, N], bf16)  # mvw * m, replicated
    mvwms2_d = ein("mvwms2", [48, SH], bf16)   # -2 * (mvw*m) shard cols
    uro_d = ein("uro", [128, 17 * NJ], bf16)   # [U|1] rows per 128-chunk
    rro0_d = ein("rro0", [128, 17], bf16)      # [R_s|1] rows
    rro1_d = ein("rro1", [64, 17], bf16)
    urs0_d = ein("urs0", [128, H])       # U_s rows fp32 (assembly)
    urs1_d = ein("urs1", [64, H])
    rrs0_d = ein("rrs0", [128, H])       # R_s rows fp32 (assembly)
    rrs1_d = ein("rrs1", [64, H])
    dds0_d = ein("dds0", [128, 32])      # host-computed dissipated term
    dds1_d = ein("dds1", [64, 32])
    W1q_d = ein("W1q", [H, 32])
    identb_d = ein("identb", [128, 128], bf16)
    identf_d = ein("identf", [128, 128])

    dp_d = nc.dram_tensor("dp_s", [SH, 32], f32, kind="ExternalOutput")

    with tile.TileContext(nc) as tc:
        with (
            tc.tile_pool(name="const", bufs=1) as cp,
            tc.tile_pool(name="work", bufs=3) as wp,
            tc.tile_pool(name="big", bufs=1) as wp1,
            tc.tile_pool(name="dram", bufs=1, space="DRAM") as drp,
        ):
            def load(eng, d, shape, tag, dt=f32):
                t = cp.tile(shape, dt, tag=tag)
                eng.dma_start(t[:], d[:])
                return t

            # inputs spread across engine DMA queues, first-use first
            Slhs = load(nc.scalar, Slhs_d, [18, SH], "Slhs", f32r)
            Srhs = cp.tile([18, N], f32r, tag="Srhs")
            for k in range(NJ3):
                nc.sync.dma_start(Srhs[:, k * 512:(k + 1) * 512],
                                  Srhs_d[:, k * 512:(k + 1) * 512])
            mvwms2 = load(nc.gpsimd, mvwms2_d, [48, SH], "mvwms2", bf16)
            mvwm = cp.tile([48, N], bf16, tag="mvwm")
            for k in range(NJ3):
                nc.scalar.dma_start(mvwm[:, k * 512:(k + 1) * 512],
                                    mvwm_d[:, k * 512:(k + 1) * 512])
            rro0 = load(nc.gpsimd, rro0_d, [128, 17], "rro0", bf16)
            rro1 = load(nc.gpsimd, rro1_d, [64, 17], "rro1", bf16)
            identb = load(nc.gpsimd, identb_d, [128, 128], "identb", bf16)
            uro = load(nc.gpsimd, uro_d, [128, 17 * NJ], "uro", bf16)
            urs0 = load(nc.gpsimd, urs0_d, [128, H], "urs0")
            urs1 = load(nc.gpsimd, urs1_d, [64, H], "urs1")
            rrs0 = load(nc.gpsimd, rrs0_d, [128, H], "rrs0")
            rrs1 = load(nc.gpsimd, rrs1_d, [64, H], "rrs1")
            dds0 = load(nc.gpsimd, dds0_d, [128, 32], "dds0")
            dds1 = load(nc.gpsimd, dds1_d, [64, 32], "dds1")
            W1q = load(nc.gpsimd, W1q_d, [H, 32], "W1q")
            identf = load(nc.gpsimd, identf_d, [128, 128], "identf")

            # C tiles, bf16, [i, j] layout
            ct0 = cp.tile([128, N], bf16, tag="ct0")
            ct1 = cp.tile([64, N], bf16, tag="ct1")

            P_dram = drp.tile([N, 17], f32)
            P_out = drp.tile([N, 17], f32)

            TILES = [(it, off, w, k) for k in range(NJ3)
                     for it, (off, w) in enumerate(ITILES)]

            # PSUM pools hand out full 2KB banks; sub-tiles are sliced out so
            # no extra banks are burned per distinct shape. 2+1+1+2+2 = 8.
            with (
                tc.tile_pool(name="psS", bufs=2, space="PSUM") as psS,
                tc.tile_pool(name="psM", bufs=1, space="PSUM") as psM,
                tc.tile_pool(name="psP", bufs=2, space="PSUM") as psP,
                tc.tile_pool(name="psT", bufs=1, space="PSUM") as psT,
                tc.tile_pool(name="psB", bufs=2, space="PSUM") as psB,
            ):
                # ---- PE: dependency-free dummy matmul first: the runtime's
                # pre-collective bootstrap BARRIER (~40us channel setup) is
                # anchored to early PE-queue progress, so fire it immediately
                dmy = wp.tile([1, 1], f32, tag="dmy")
                nc.vector.memset(dmy[:], 0.0)
                dps_ = psP.tile([128, 512], f32, tag="p")
                nc.tensor.matmul(dps_[0:1, 0:1], dmy[:], dmy[:],
                                 start=True, stop=True)

                # ---- PE: S matmuls; copy S to SBUF so the 2 PSUM banks rotate
                S_sb = []
                for it, off, w, k in TILES:
                    sb_ = psS.tile([128, 512], f32, tag="s")
                    nc.tensor.matmul(sb_[0:w, 0:512], Slhs[:, off:off + w],
                                     Srhs[:, k * 512:(k + 1) * 512],
                                     start=True, stop=True)
                    ss = wp1.tile([w, 512], f32, tag=f"ss{it}{k}")
                    nc.vector.tensor_copy(ss[:], sb_[0:w, 0:512])
                    S_sb.append(ss)

                # ---- PE: mask matmuls (bf16; PSUM-resident until sm) ----
                mk_ps = []
                for it, off, w, k in TILES:
                    mb_ = psM.tile([128, 512], f32, tag="m")
                    nc.tensor.matmul(mb_[0:w, 0:512], mvwms2[:, off:off + w],
                                     mvwm[:, k * 512:(k + 1) * 512],
                                     start=True, stop=True)
                    mk_ps.append(mb_[0:w, 0:512])

                # ---- ACT chain, all on the natural_log_exp table ----
                e_sb = []
                for n_, (it, off, w, k) in enumerate(TILES):
                    ee = wp1.tile([w, 512], f32, tag=f"ee{it}{k}")
                    nc.scalar.activation(ee[:], S_sb[n_][:], AF.Exp, scale=-1.0)
                    e_sb.append(ee)
                l_sb = []
                for n_, (it, off, w, k) in enumerate(TILES):
                    ll = wp1.tile([w, 512], f32, tag=f"ll{it}{k}")
                    nc.scalar.activation(ll[:], e_sb[n_][:], AF.Ln, bias=1.0)
                    l_sb.append(ll)
                # DVE: d = S + ln(1+exp(-S))  (exact softplus; S >= 0)
                d_sb = []
                for n_, (it, off, w, k) in enumerate(TILES):
                    dd = wp1.tile([w, 512], f32, tag=f"dd{it}{k}")
                    nc.vector.tensor_add(dd[:], S_sb[n_][:], l_sb[n_][:])
                    d_sb.append(dd)
                # lnd batch directly follows the l1 Ln batch (same table)
                ln_sb = []
                for n_, (it, off, w, k) in enumerate(TILES):
                    ld = wp1.tile([w, 512], f32, tag=f"ld{it}{k}")
                    nc.scalar.activation(ld[:], d_sb[n_][:], AF.Ln)
                    ln_sb.append(ld)
                # em = exp(-d) = 1 - sigmoid(S); p3 = d^-3 (one Exp batch)
                em_sb = []
                for n_, (it, off, w, k) in enumerate(TILES):
                    em = wp1.tile([w, 512], f32, tag=f"em{it}{k}")
                    nc.scalar.activation(em[:], d_sb[n_][:], AF.Exp, scale=-1.0)
                    em_sb.append(em)
                p3_sb = []
                for n_, (it, off, w, k) in enumerate(TILES):
                    p3 = wp1.tile([w, 512], f32, tag=f"p3{it}{k}")
                    nc.scalar.activation(p3[:], ln_sb[n_][:], AF.Exp, scale=-3.0)
                    p3_sb.append(p3)
                # sm = (em - 1) * mask = sigmoid(S) * |mask|  (mask shipped
                # negated, so the sign comes out right)
                sm_sb = []
                for n_, (it, off, w, k) in enumerate(TILES):
                    sm = wp1.tile([w, 512], f32, tag=f"sm{it}{k}")
                    nc.vector.scalar_tensor_tensor(
                        sm[:], em_sb[n_][:], -1.0, mk_ps[n_],
                        op0=OP.add, op1=OP.mult)
                    sm_sb.append(sm)

                # ---- DVE: C = ((d-2)*d^-3) * sm -> bf16; P matmuls per
                # 128-chunk -> DRAM. B transposes/accum deferred into the
                # collective window. ----
                for k in range(NJ3):
                    for it, (off, w) in enumerate(ITILES):
                        n_ = 2 * k + it
                        ct = (ct0, ct1)[it]
                        t_ = wp.tile([w, 512], f32, tag=f"t{it}")
                        nc.vector.scalar_tensor_tensor(
                            t_[:], d_sb[n_][:], -2.0, p3_sb[n_][:],
                            op0=OP.add, op1=OP.mult)
                        nc.vector.tensor_mul(ct[:, k * 512:(k + 1) * 512],
                                             t_[:], sm_sb[n_][:])
                    for sub in range(4):
                        jc = 4 * k + sub
                        pb_ = psP.tile([128, 512], f32, tag="p")
                        pp = pb_[0:128, 0:17]
                        nc.tensor.matmul(pp, ct0[:, jc * 128:(jc + 1) * 128],
                                         rro0[:], start=True, stop=False)
                        nc.tensor.matmul(pp, ct1[:, jc * 128:(jc + 1) * 128],
                                         rro1[:], start=False, stop=True)
                        psb_ = wp.tile([128, 17], f32, tag="psb")
                        nc.vector.tensor_copy(psb_[:], pp)
                        eng = (nc.sync, nc.gpsimd, nc.sync, nc.gpsimd)[sub]
                        eng.dma_start(P_dram[jc * 128:(jc + 1) * 128, :],
                                      psb_[:])

                nc.gpsimd.collective_compute(
                    "AllToAll",
                    mybir.AluOpType.bypass,
                    replica_groups=[list(range(NCORES))],
                    ins=[P_dram.opt()],
                    outs=[P_out.opt()],
                )

                # ---- B = C @ [U|1] via bf16 PE transposes (inside A2A) ----
                bb0 = psB.tile([128, 512], f32, tag="b")
                bb1 = psB.tile([128, 512], f32, tag="b")
                bp0 = bb0[0:128, 0:17]
                bp1 = bb1[0:64, 0:17]
                for jc in range(NJ):
                    for it, (off, w) in enumerate(ITILES):
                        ct = (ct0, ct1)[it]
                        bp = (bp0, bp1)[it]
                        tb_ = psT.tile([128, 1024], bf16, tag="ct")
                        tp = tb_[0:128, 0:w]
                        nc.tensor.transpose(tp, ct[:, jc * 128:(jc + 1) * 128],
                                            identb[0:w, 0:w])
                        tsbb = wp.tile([128, w], bf16, tag=f"tsbb{it}")
                        nc.vector.tensor_copy(tsbb[:], tp)
                        nc.tensor.matmul(bp, tsbb[:],
                                         uro[:, jc * 17:(jc + 1) * 17],
                                         start=(jc == 0), stop=(jc == NJ - 1))
                bsb = []
                for it, (off, w) in enumerate(ITILES):
                    bs = wp.tile([w, 17], f32, tag=f"bsb{it}")
                    nc.vector.tensor_copy(bs[:], (bp0, bp1)[it])
                    bsb.append(bs)

                # ---- tail: gather + tree-sum the 8 P_out slabs, assemble ----
                acc0 = wp.tile([128, 8 * 17], f32, tag="acc0")
                acc1w = wp.tile([64, 8 * 17], f32, tag="acc1w")
                for c in range(NCORES):
                    nc.sync.dma_start(
                        acc0[:, c * 17:(c + 1) * 17],
                        P_out[c * SH:c * SH + 128, :])
                    nc.scalar.dma_start(
                        acc1w[:, c * 17:(c + 1) * 17],
                        P_out[c * SH + 128:(c + 1) * SH, :])
                for c in range(1, NCORES):
                    nc.vector.tensor_add(acc0[:, 0:17], acc0[:, 0:17],
                                         acc0[:, c * 17:c * 17 + 17])
                    nc.gpsimd.tensor_add(acc1w[:, 0:17], acc1w[:, 0:17],
                                         acc1w[:, c * 17:c * 17 + 17])
                for it, (off, w) in enumerate(ITILES):
                    acc = (acc0, acc1w)[it]
                    urs = (urs0, urs1)[it]
                    rrs = (rrs0, rrs1)[it]
                    bs = bsb[it]
                    # A - B = (urs*cc - P16) + (rrs*bc - bs16)
                    a_t = wp.tile([w, H], f32, tag="a_t")
                    nc.vector.scalar_tensor_tensor(
                        a_t[:], urs[:], acc[:, H:H + 1], acc[:, 0:H],
                        op0=OP.mult, op1=OP.subtract)
                    b_t = wp.tile([w, H], f32, tag="b_t")
                    nc.vector.scalar_tensor_tensor(
                        b_t[:], rrs[:], bs[:, H:17], bs[:, 0:H],
                        op0=OP.mult, op1=OP.subtract)
                    d_t = wp.tile([w, H], f32, tag="d_t")
                    nc.vector.tensor_add(d_t[:], a_t[:], b_t[:])
                    mb_ = psM.tile([128, 512], f32, tag="m")
                    dtp = mb_[0:H, 0:w]
                    nc.tensor.transpose(dtp, d_t[:], identf[0:w, 0:w])
                    dts = wp.tile([H, w], f32, tag="dts")
                    nc.vector.tensor_copy(dts[:], dtp)
                    hb_ = psP.tile([128, 512], f32, tag="p")
                    hq = hb_[0:w, 0:32]
                    nc.tensor.matmul(hq, dts[:], W1q[:], start=True, stop=True)
                    dpo = wp.tile([w, 32], f32, tag="dpo")
                    # dp = -(hq + dds) = (hq * -1) - dds
                    nc.vector.scalar_tensor_tensor(
                        dpo[:], hq, -1.0, (dds0, dds1)[it][:],
                        op0=OP.mult, op1=OP.subtract)
                    eng2 = (nc.sync, nc.scalar)[it]
                    eng2.dma_start(dp_d[off:off + w, :], dpo[:])

    nc.finalize()
    return nc


def _prepare_in_maps(v, e, m, p, q, mvw, W_T, W1_w, W1_b, W_F):
    f32 = np.float32
    v, m, p, q, mvw = (np.asarray(x, f32) for x in (v, m, p, q, mvw))
    W_T, W1_w, W1_b, W_F = (np.asarray(x, f32) for x in (W_T, W1_w, W1_b, W_F))
    import ml_dtypes
    bf16 = ml_dtypes.bfloat16

    vs = (1.0 / (1.0 + np.exp(-v))).astype(f32)
    vq = np.concatenate([vs, q], axis=1)                    # [N,96]
    R = (vq @ W1_w.T).astype(f32)                           # [N,16]
    U = (R + W1_b[None, :]).astype(f32)
    rn2 = (R * R).sum(axis=1).astype(f32)
    un2 = (U * U).sum(axis=1).astype(f32)
    ones = np.ones((N,), f32)

    Srhs = np.ascontiguousarray(np.vstack([U.T, ones[None, :], un2[None, :]]))
    Slhs_full = np.vstack([-2.0 * R.T, rn2[None, :], ones[None, :]])
    mvwm = np.ascontiguousarray(mvw * m[:, 0][None, :])              # [48,N]

    # kinetic (dq) and dissipated branches: pure input functions, done here.
    def sp_sig(z):
        pw = np.logaddexp(0.0, z)
        sg = 1.0 / (1.0 + np.exp(-z))
        return pw * sg

    mi2 = 2.0 / m                                           # [N,1]
    zT = np.concatenate([vs, p], axis=1) @ W_T.T            # [N,16]
    dq_full = (mi2 * sp_sig(zT)) @ W_T[:, VD:]              # [N,32]
    zF = p @ W_F.T
    dds_full = (mi2 * sp_sig(zF)) @ W_F                     # [N,32]

    uro = np.empty((128, 17 * NJ), f32)
    for jc in range(NJ):
        uro[:, jc * 17:jc * 17 + H] = U[jc * 128:(jc + 1) * 128, :]
        uro[:, jc * 17 + H] = 1.0

    shared = {
        "Srhs": Srhs,
        "mvwm": mvwm.astype(bf16),
        "uro": uro.astype(bf16),
        "W1q": np.ascontiguousarray(W1_w[:, VD:]),
        "identb": np.eye(128, dtype=f32).astype(bf16),
        "identf": np.eye(128, dtype=f32),
    }
    in_maps = []
    for c in range(NCORES):
        sl = slice(c * SH, (c + 1) * SH)
        Rs, Us = R[sl], U[sl]
        rro = np.empty((SH, 17), f32)
        rro[:, 0:H] = Rs
        rro[:, H] = 1.0
        dds_s = dds_full[sl].astype(f32)
        in_maps.append({
            **shared,
            "Slhs": np.ascontiguousarray(Slhs_full[:, sl]),
            "mvwms2": np.ascontiguousarray(-2.0 * mvwm[:, sl]).astype(bf16),
            "rro0": np.ascontiguousarray(rro[0:128]).astype(bf16),
            "rro1": np.ascontiguousarray(rro[128:]).astype(bf16),
            "urs0": np.ascontiguousarray(Us[0:128]),
            "urs1": np.ascontiguousarray(Us[128:]),
            "rrs0": np.ascontiguousarray(Rs[0:128]),
            "rrs1": np.ascontiguousarray(Rs[128:]),
            "dds0": np.ascontiguousarray(dds_s[0:128]),
            "dds1": np.ascontiguousarray(dds_s[128:]),
        })
    return in_maps, dq_full.astype(f32)


def kernel(v, e, m, p, q, mvw, W_T, W1_w, W1_b, W_F):
    from concourse.bass_utils import run_bass_kernel_spmd

    in_maps, dq_full = _prepare_in_maps(v, e, m, p, q, mvw, W_T, W1_w, W1_b, W_F)

    if "nc" not in _CACHE:
        _CACHE["nc"] = _build_nc()
    nc = _CACHE["nc"]

    trace = bool(os.environ.get("BASS_KERNEL_TRACE"))
    if trace:
        try:
            from antenv.axon_hooks import get_axon_ntff_profile_hook  # noqa: F401
        except ImportError:
            trace = False
    res = run_bass_kernel_spmd(nc, in_maps, list(range(NCORES)), trace=trace)
    if trace and res.exec_time_ns is not None:
        print(f"HW exec time: {res.exec_time_ns} ns")

    dp = np.concatenate([res.results[c]["dp_s"] for c in range(NCORES)], axis=0)
    return dp, dq_full
